# revision 11
# baseline (speedup 1.0000x reference)
"""Single-head attention Trainium2 kernel (batch=8 data-parallel over 8 cores).

Reference (per batch element): out = softmax((x Wq)(x Wk)^T / 8) (x Wv).

Strategy: fp8-e4m3 residual (hi+lo) arithmetic on the PE at DoubleRow rate
(0.5 cycles/row, 256-deep contraction per instruction), with all first-order
quantization error cancelled by 3-pass matmuls (hi*hi + lo*hi + hi*lo).
Measured end-to-end absmax_rel ~3e-3 (gate 2e-2).

Algebraic restructure: scores = x M x^T with M = Wq Wk^T precomputed on the
host (exact f32), so the device never materializes Q or K:
    G'^T = M^T-contraction vs x^T     [d', s]
    scoresT = x^T-contraction vs G'^T [ks, q]
    expT = exp(scores/8 - 5.5)        (global exp bias cancels in softmax)
    A^T  = x-contraction vs expT      [d, q]   (= (attn @ x)^T unnormalized)
    out  = A-contraction vs Wv, * 1/rowsum
Host also pre-splits M and Wv into hi/lo e4m3 pair-tile layouts (pure weight
preprocessing; x-dependent work all happens on device).

Power-of-2 scales keep every rescale exact: sx=16, sM=2048, sG=64, sWv=1024,
sA=1/4; hi and lo share one scale so all 3 residual passes accumulate in a
single PSUM group.
"""

import sys

sys.path.insert(0, "/opt/trn_rl_repo")

from contextlib import ExitStack

import numpy as np
import ml_dtypes

import concourse.bass as bass
import concourse.mybir as mybir
from concourse import bacc
from concourse.tile import TileContext
from concourse.masks import make_identity

F32 = mybir.dt.float32
F32R = mybir.dt.float32r
E4 = mybir.dt.float8e4
U8 = mybir.dt.uint8
DR = mybir.MatmulPerfMode.DoubleRow
EXP = mybir.ActivationFunctionType.Exp
COPY = mybir.ActivationFunctionType.Copy
MULT = mybir.AluOpType.mult
SUBTRACT = mybir.AluOpType.subtract

E4NP = ml_dtypes.float8_e4m3

S, D, O = 2048, 1024, 1024
SX = 16.0          # x scale
SM = 2048.0        # M scale
SG = 64.0          # G' scale
SV = 1024.0        # Wv scale
SA = 0.25          # A scale
BIAS = -5.5        # exp bias (cancels in softmax), keeps e4m3 in range
C_G = SG / (SX * SM)        # 2^-9  : psum(G'*sx*sM) -> G'*sG
C_E = (1.0 / 8.0) / (SX * SG)  # 2^-13: psum(scores*sx*sG) -> scores/8
C_A = SA / SX               # 2^-6  : psum(A*sx) -> A*sA
C_O = 1.0 / (SA * SV)       # 2^-8  : psum(out*sA*sV) -> out (pre-rowsum)


def build_attn(reps=1):
    """Bass module for one core: x[S,D] f32, mprep/vprep fp8 -> out[S,O] f32."""
    KC = S // 128      # 16 ks-chunks
    KP = KC // 2       # 8 ks-pairs
    DP = D // 256      # 4 d-pairs
    NSB = S // 512     # 4 s-blocks (also the q-blocks)

    nc = bacc.Bacc("TRN2", target_bir_lowering=False, debug=False)
    x_in = nc.dram_tensor("x", [S, D], F32, kind="ExternalInput")
    m_in = nc.dram_tensor("mprep", [2, DP, 128, 2, D], U8, kind="ExternalInput")
    v_in = nc.dram_tensor("vprep", [2, DP, 128, 2, O], U8, kind="ExternalInput")
    out_d = nc.dram_tensor("out", [S, O], F32, kind="ExternalOutput")

    with TileContext(nc) as tc:
      for _rep in range(reps):
        top = ExitStack()
        const_pool = top.enter_context(tc.tile_pool(name="constp", bufs=4))
        w_pool = top.enter_context(tc.tile_pool(name="wp", bufs=4 * DP))
        xr_pool = top.enter_context(tc.tile_pool(name="xrp", bufs=2 * KP))
        xt_pool = top.enter_context(tc.tile_pool(name="xtp", bufs=2 * DP * NSB))
        gt_pool = top.enter_context(tc.tile_pool(name="gtp", bufs=2 * DP * NSB))

        ident_f = const_pool.tile([128, 128], F32, tag="identf")
        make_identity(nc, ident_f)
        ident_r = const_pool.tile([128, 128], F32R, tag="identr")
        nc.vector.tensor_copy(out=ident_r, in_=ident_f)
        ones8 = const_pool.tile([128, 2, 1], E4, tag="ones8")
        nc.gpsimd.memset(ones8, 1.0)
        bias_t = const_pool.tile([128, 1], F32, tag="biast")
        nc.gpsimd.memset(bias_t, BIAS)

        # -------- weight tiles (DMAs deferred into the s-block loop so the
        # first x rows win the queue; vprep is only needed in phase 4) ----
        mst = [[None] * DP for _ in range(2)]   # [hl][dp] -> [128, 2, D]
        wvt = [[None] * DP for _ in range(2)]
        for hl in range(2):
            for dp in range(DP):
                mst[hl][dp] = w_pool.tile([128, 2, D], E4, tag="mst",
                                          bufs=2 * DP, name=f"mst_{hl}_{dp}")
                wvt[hl][dp] = w_pool.tile([128, 2, O], E4, tag="wvt",
                                          bufs=2 * DP, name=f"wvt_{hl}_{dp}")

        # -------- phases 1-3 interleaved per s-block --------
        # load + row-split (ACT hi / DVE lo), f32r transposes with split on
        # PSUM eviction, then G'^T DoubleRow matmuls with split on eviction.
        xr8 = [[None] * KP for _ in range(2)]   # [hl][kp] -> [128, 2, D]
        for hl in range(2):
            for kp in range(KP):
                xr8[hl][kp] = xr_pool.tile([128, 2, D], E4, tag="xr",
                                           bufs=2 * KP, name=f"xr_{hl}_{kp}")
        with ExitStack() as ph1:
            xn_pool = ph1.enter_context(tc.tile_pool(name="xnp", bufs=8))
            xt8 = [[[None] * NSB for _ in range(DP)] for _ in range(2)]
            gt8 = [[[None] * NSB for _ in range(DP)] for _ in range(2)]
            pst = ph1.enter_context(tc.tile_pool(name="pst", bufs=2, space="PSUM"))
            psg = ph1.enter_context(tc.tile_pool(name="psg", bufs=3, space="PSUM"))
            def g_stage(sb):
                for dpc in range(D // 128):
                    pg = psg.tile([128, 512], F32, tag="pg", bufs=3)
                    n = 0
                    for (hm, hx) in ((0, 0), (0, 1), (1, 0)):
                        for dp in range(DP):
                            nc.tensor.matmul(
                                pg,
                                mst[hm][dp][:, :, dpc * 128:(dpc + 1) * 128],
                                xt8[hx][dp][sb],
                                start=(n == 0), stop=(n == 3 * DP - 1),
                                perf_mode=DR)
                            n += 1
                    if gt8[0][dpc // 2][sb] is None:
                        gt8[0][dpc // 2][sb] = gt_pool.tile(
                            [128, 2, 512], E4, tag="gt",
                            bufs=2 * DP * NSB, name=f"gt_0_{dpc}_{sb}")
                        gt8[1][dpc // 2][sb] = gt_pool.tile(
                            [128, 2, 512], E4, tag="gt",
                            bufs=2 * DP * NSB, name=f"gt_1_{dpc}_{sb}")
                    dh = gt8[0][dpc // 2][sb]
                    dl = gt8[1][dpc // 2][sb]
                    nc.scalar.activation(out=dh[:, dpc % 2, :], in_=pg,
                                         func=COPY, scale=C_G)
                    nc.vector.scalar_tensor_tensor(
                        out=dl[:, dpc % 2, :], in0=pg, scalar=C_G,
                        in1=dh[:, dpc % 2, :], op0=MULT, op1=SUBTRACT)

            for sb in range(NSB):
                xns = []
                for ss in range(4):
                    kc = sb * 4 + ss
                    xn = xn_pool.tile([128, D], F32R, tag="xn", bufs=8)
                    dma_eng = nc.sync if kc % 2 == 0 else nc.gpsimd
                    dma_eng.dma_start(
                        out=xn, in_=x_in[kc * 128:(kc + 1) * 128, :].bitcast(F32R))
                    xns.append(xn)
                    kp, h = kc // 2, kc % 2
                    nc.scalar.activation(out=xr8[0][kp][:, h, :],
                                         in_=xn.bitcast(F32), func=COPY, scale=SX)
                    nc.vector.scalar_tensor_tensor(
                        out=xr8[1][kp][:, h, :], in0=xn.bitcast(F32), scalar=SX,
                        in1=xr8[0][kp][:, h, :], op0=MULT, op1=SUBTRACT)
                if sb == 0:
                    for hl in range(2):
                        for dp in range(DP):
                            nc.sync.dma_start(out=mst[hl][dp],
                                              in_=m_in[hl, dp].bitcast(E4))
                elif sb == 1:
                    for hl in range(2):
                        for dp in range(DP):
                            nc.gpsimd.dma_start(out=wvt[hl][dp],
                                                in_=v_in[hl, dp].bitcast(E4))
                # transposes: x[sb-block rows] -> xT, split hi/lo on eviction
                for dc in range(D // 128):
                    pt = pst.tile([128, 512], F32R, tag="pt", bufs=2)
                    for ss in range(4):
                        nc.tensor.transpose(
                            pt[:, ss * 128:(ss + 1) * 128],
                            xns[ss][:, dc * 128:(dc + 1) * 128],
                            ident_r,
                        )
                    if xt8[0][dc // 2][sb] is None:
                        for hl in range(2):
                            xt8[hl][dc // 2][sb] = xt_pool.tile(
                                [128, 2, 512], E4, tag="xt",
                                bufs=2 * DP * NSB, name=f"xt_{hl}_{dc}_{sb}")
                    ptf = pt.bitcast(F32)
                    dh = xt8[0][dc // 2][sb]
                    dl = xt8[1][dc // 2][sb]
                    nc.scalar.activation(out=dh[:, dc % 2, :], in_=ptf,
                                         func=COPY, scale=SX)
                    nc.vector.scalar_tensor_tensor(
                        out=dl[:, dc % 2, :], in0=ptf, scalar=SX,
                        in1=dh[:, dc % 2, :], op0=MULT, op1=SUBTRACT)
                # G' for the previous s-block overlaps this block's
                # transpose evictions (PE never waits on ACT/DVE)
                if sb >= 1:
                    g_stage(sb - 1)
            g_stage(NSB - 1)

        # -------- phase 4: attention per q-block (qb = s-block of 512) ----
        with ExitStack() as ph4:
            e_pool = ph4.enter_context(tc.tile_pool(name="ep", bufs=4 * KP))
            e32_pool = ph4.enter_context(tc.tile_pool(name="e32p", bufs=4))
            a_pool = ph4.enter_context(tc.tile_pool(name="ap", bufs=4 * DP))
            small_pool = ph4.enter_context(tc.tile_pool(name="smallp", bufs=16))
            outs_pool = ph4.enter_context(tc.tile_pool(name="outsp", bufs=3))
            pcs = ph4.enter_context(tc.tile_pool(name="pcs", bufs=3, space="PSUM"))
            pca = ph4.enter_context(tc.tile_pool(name="pca", bufs=2, space="PSUM"))
            pco = ph4.enter_context(tc.tile_pool(name="pco", bufs=2, space="PSUM"))
            pcr = ph4.enter_context(tc.tile_pool(name="pcr", bufs=1, space="PSUM"))

            def scores_stage(qb):
                """scoresT + exp splits for q-block qb -> e8 tiles."""
                e8 = [[None] * KP for _ in range(2)]
                for kc in range(KC):
                    ps = pcs.tile([128, 512], F32, tag="ps", bufs=3)
                    sbk, ss = kc // 4, kc % 4
                    n = 0
                    for (ha, hb) in ((0, 0), (0, 1), (1, 0)):
                        for dp in range(DP):
                            nc.tensor.matmul(
                                ps,
                                xt8[ha][dp][sbk][:, :, ss * 128:(ss + 1) * 128],
                                gt8[hb][dp][qb],
                                start=(n == 0), stop=(n == 3 * DP - 1),
                                perf_mode=DR)
                            n += 1
                    e32 = e32_pool.tile([128, 512], F32, tag="e32", bufs=4)
                    nc.scalar.activation(out=e32, in_=ps, func=EXP,
                                         bias=bias_t, scale=C_E)
                    kp, h = kc // 2, kc % 2
                    if h == 0:
                        e8[0][kp] = e_pool.tile([128, 2, 512], E4, tag="e8",
                                                bufs=4 * KP,
                                                name=f"e8h_{qb}_{kp}")
                        e8[1][kp] = e_pool.tile([128, 2, 512], E4, tag="e8",
                                                bufs=4 * KP,
                                                name=f"e8l_{qb}_{kp}")
                    nc.gpsimd.tensor_copy(out=e8[0][kp][:, h, :], in_=e32)
                    nc.gpsimd.tensor_sub(out=e8[1][kp][:, h, :], in0=e32,
                                         in1=e8[0][kp][:, h, :])
                return e8

            def attend_stage(qb, e8):
                """A^T, rowsums, out for q-block qb."""
                a8 = [[None] * DP for _ in range(2)]
                for dc in range(D // 128):
                    pa = pca.tile([128, 512], F32, tag="pa", bufs=2)
                    n = 0
                    for (hx, he) in ((0, 0), (0, 1), (1, 0)):
                        for kp in range(KP):
                            nc.tensor.matmul(
                                pa,
                                xr8[hx][kp][:, :, dc * 128:(dc + 1) * 128],
                                e8[he][kp],
                                start=(n == 0), stop=(n == 3 * KP - 1),
                                perf_mode=DR)
                            n += 1
                    dp, h = dc // 2, dc % 2
                    if h == 0:
                        a8[0][dp] = a_pool.tile([128, 2, 512], E4, tag="a8",
                                                bufs=4 * DP,
                                                name=f"a8h_{qb}_{dp}")
                        a8[1][dp] = a_pool.tile([128, 2, 512], E4, tag="a8",
                                                bufs=4 * DP,
                                                name=f"a8l_{qb}_{dp}")
                    nc.scalar.activation(out=a8[0][dp][:, h, :], in_=pa,
                                         func=COPY, scale=C_A)
                    nc.vector.scalar_tensor_tensor(
                        out=a8[1][dp][:, h, :], in0=pa, scalar=C_A,
                        in1=a8[0][dp][:, h, :], op0=MULT, op1=SUBTRACT)
                # rowsums (over all ks) per q-chunk -> 1/(rowsum) * C_O
                rcs = []
                for qc in range(4):
                    pr = pcr.tile([128, 1], F32, tag="pr", bufs=1)
                    n = 0
                    for he in range(2):
                        for kp in range(KP):
                            nc.tensor.matmul(
                                pr,
                                e8[he][kp][:, :, qc * 128:(qc + 1) * 128],
                                ones8,
                                start=(n == 0), stop=(n == 2 * KP - 1),
                                perf_mode=DR)
                            n += 1
                    rc = small_pool.tile([128, 1], F32, tag="rc", bufs=16,
                                         name=f"rc_{qb}_{qc}")
                    nc.vector.reciprocal(out=rc, in_=pr)
                    rc2 = small_pool.tile([128, 1], F32, tag="rc2", bufs=16,
                                          name=f"rc2_{qb}_{qc}")
                    nc.scalar.activation(out=rc2, in_=rc, func=COPY, scale=C_O)
                    rcs.append(rc2)
                # out = A-contraction vs Wv, normalized
                for qc in range(4):
                    for oh in range(2):
                        po = pco.tile([128, 512], F32, tag="po", bufs=2)
                        n = 0
                        for (ha, hv) in ((0, 0), (0, 1), (1, 0)):
                            for dp in range(DP):
                                nc.tensor.matmul(
                                    po,
                                    a8[ha][dp][:, :, qc * 128:(qc + 1) * 128],
                                    wvt[hv][dp][:, :, oh * 512:(oh + 1) * 512],
                                    start=(n == 0), stop=(n == 3 * DP - 1),
                                    perf_mode=DR)
                                n += 1
                        os_ = outs_pool.tile([128, 512], F32, tag="outs",
                                             bufs=3)
                        nc.vector.tensor_scalar_mul(out=os_, in0=po,
                                                    scalar1=rcs[qc])
                        nc.sync.dma_start(
                            out=out_d[qb * 512 + qc * 128:
                                      qb * 512 + (qc + 1) * 128,
                                      oh * 512:(oh + 1) * 512],
                            in_=os_)

            # software pipeline: scores(qb) runs ahead of attend(qb-1)
            prev = None
            for qb in range(NSB):
                e8 = scores_stage(qb)
                if prev is not None:
                    attend_stage(qb - 1, prev)
                prev = e8
            attend_stage(NSB - 1, prev)

        top.close()

    nc.compile()
    return nc


def _split_np(a, scale):
    hi = (a * scale).astype(E4NP)
    lo = ((a * scale) - hi.astype(np.float32)).astype(E4NP)
    return hi, lo


def _prep_pairs(t):
    """[1024 d, 1024 c] fp8 -> [dp, p, h, c] with d = dp*256 + h*128 + p."""
    return np.ascontiguousarray(t.reshape(4, 2, 128, 1024).transpose(0, 2, 1, 3))


def prepare_weights(w):
    """Host-side weight prep: M = Wq Wk^T (f32) and hi/lo e4m3 pair tiles."""
    M = (w[0].astype(np.float64) @ w[1].T.astype(np.float64)).astype(np.float32)
    Mh, Ml = _split_np(M, SM)
    Vh, Vl = _split_np(w[2], SV)
    mprep = np.stack([_prep_pairs(Mh), _prep_pairs(Ml)])
    vprep = np.stack([_prep_pairs(Vh), _prep_pairs(Vl)])
    return mprep.view(np.uint8), vprep.view(np.uint8)


_NC_CACHE = {}


def _get_nc():
    if "full" not in _NC_CACHE:
        _NC_CACHE["full"] = build_attn()
    return _NC_CACHE["full"]


def kernel(**inputs):
    """Full-input entry point: x [8, 2048, 1024], kernel [3, 1024, 1024]."""
    from concourse.bass_utils import run_bass_kernel_spmd

    x = np.ascontiguousarray(inputs["x"], dtype=np.float32)
    w = np.ascontiguousarray(inputs["kernel"], dtype=np.float32)
    B = x.shape[0]
    mprep, vprep = prepare_weights(w)
    nc = _get_nc()
    in_maps = [{"x": x[b], "mprep": mprep, "vprep": vprep} for b in range(B)]
    res = run_bass_kernel_spmd(nc, in_maps, core_ids=list(range(B)))
    return np.stack([res.results[b]["out"] for b in range(B)], axis=0)
